# revision 27
# baseline (speedup 1.0000x reference)
"""3x3 valid conv via 1D Winograd F(4,3) along H, on 8 Trainium2 cores,
with the output (A^T) combine moved to the HOST.

x: (16, 128, 64, 64) f32, weights: (256, 128, 3, 3) f32
-> out: (16, 256, 62, 62) f32

Data-parallel, 2 images per core. Interpolation points {0, 1, -1, 1/2, -2, inf}
(chosen to minimize bf16 transform error; classic {0,±1,±2} overflows the
2e-2 gate). 16 row-tiles of 4 output rows (tile 15 overlaps: rows 58-61).

Per tile: 6 nu x 3 kj = 18 matmuls per 4 output rows (vs 12 per 2 rows for
F(2,3)) -> 25% less PE work, and the device does NO output combine at all:
each PSUM bank M[nu] is copied to SBUF as bf16 (DVE/Act alternating) and
DMA'd out; the host applies y = A^T M in f32. This removes the DVE
tensor_tensor wall (the F(2,3) kernel's combine trailed the PE stream) and
cuts store bytes to 6.1MB bf16.

V transform (B^T, host, f64->bf16) and G weight transform also on host.
Three HWDGE rings: sync = weights + bulk V, Act = first V chunks + group
stores, gpsimd = two mid-startup V chunks (its DSP compute stays idle).
8 fp32 prewarm matmuls bridge the PE seamlessly to the first real matmul:
any >1us PE gap during warmup restarts the ~5us HAM throttle warmup clock
(half-rate PE until then). M stores are group-contiguous in DRAM (5952B
per-partition elements); smaller DMA elements throttle the ring enough
that store backpressure through the staging pool paces the whole stream.
"""

import numpy as np

N_CORES = 8
IMGS_PER_CORE = 2
CIN = 128
COUT = 256
H = W = 64
OH = OW = 62
NT = 16            # row-tiles per image (4 output rows each; tile 15 at row 58)
NNU = 6
TBS = [(0, 8), (8, 8)]  # (t0, ntiles) blocks; one PSUM bank per nu holds 8 tiles

# F(4,3) with points {0, 1, -1, 1/2, -2, inf}
_AT = np.array(
    [
        [1.0, 1.0, 1.0, 1.0, 1.0, 0.0],
        [0.0, 1.0, -1.0, 0.5, -2.0, 0.0],
        [0.0, 1.0, 1.0, 0.25, 4.0, 0.0],
        [0.0, 1.0, -1.0, 0.125, -8.0, 1.0],
    ]
)
_G = np.array(
    [
        [1.0, 0.0, 0.0],
        [1 / 3, 1 / 3, 1 / 3],
        [-1 / 3, 1 / 3, -1 / 3],
        [-16 / 15, -8 / 15, -4 / 15],
        [1 / 15, -2 / 15, 4 / 15],
        [0.0, 0.0, 1.0],
    ]
)
_BT = np.array(
    [
        [1.0, -1.5, -2.0, 1.5, 1.0, 0.0],
        [0.0, -1.0, 0.5, 2.5, 1.0, 0.0],
        [0.0, 1.0, -2.5, 0.5, 1.0, 0.0],
        [0.0, -2.0, -1.0, 2.0, 1.0, 0.0],
        [0.0, 0.5, -1.0, -0.5, 1.0, 0.0],
        [0.0, 1.0, -1.5, -2.0, 1.5, 1.0],
    ]
)
_STARTS = np.array([0, 4, 8, 12, 16, 20, 24, 28, 32, 36, 40, 44, 48, 52, 56, 58])

_NC_CACHE = []


def _build():
    import concourse.bacc as bacc
    import concourse.mybir as mybir
    import concourse.tile as tile

    bf16 = mybir.dt.bfloat16
    f32 = mybir.dt.float32

    nc = bacc.Bacc("TRN2", target_bir_lowering=False, debug=False)
    # V layout: [img, cin, nu, t, col] (nu-major: per-nu chunks are 1024B
    # contiguous per partition -- 256B elements run ~10x slower on the rings)
    v_in = nc.dram_tensor(
        "v", [IMGS_PER_CORE, CIN, NNU, NT, W], bf16, kind="ExternalInput"
    ).ap()
    # w layout: [cin, (h, nu, kj, coutl)]
    w = nc.dram_tensor(
        "w", [CIN, 2 * NNU * 3 * 128], bf16, kind="ExternalInput"
    ).ap()
    # M out: [img, h, coutl, tb, nu, t8, col] -- group-contiguous so each
    # group store is a single 5952B-per-partition element (992B elements
    # run the ring at ~140-220GB/s depending on luck and can gate the
    # whole stream through staging-buffer recycling; 5952B sustains ~400)
    m_out = nc.dram_tensor(
        "m", [IMGS_PER_CORE, 2, 128, 2, NNU, 8, OW], bf16, kind="ExternalOutput"
    ).ap()

    with tile.TileContext(nc) as tc:
        with (
            tc.tile_pool(name="wp", bufs=1) as w_pool,
            tc.tile_pool(name="vp", bufs=1) as v_pool,
            tc.tile_pool(name="sp", bufs=5) as s_pool,
            tc.tile_pool(name="ps", bufs=8, space="PSUM") as ps_pool,
        ):
            # PE prewarm through the initial DMA window. memset on DVE, NOT
            # gpsimd (gpsimd boots late and tightens the HAM throttle).
            scr = w_pool.tile([CIN, 128], f32, tag="scr")
            nc.vector.memset(scr[:], 0.0)
            wrm = ps_pool.tile([128, 8, OW], f32, name="wrm", tag="p")
            for _ in range(8):
                nc.tensor.matmul(
                    wrm[:, :2, :], scr[:], scr[:, :124], start=True, stop=True
                )

            w_sb = w_pool.tile([CIN, 2 * NNU * 3 * 128], bf16, tag="w")
            vts = {
                0: v_pool.tile([CIN, NNU, NT, W], bf16, name="v0", tag="v0"),
                1: v_pool.tile([CIN, NNU, NT, W], bf16, name="v1", tag="v1"),
            }
            # Startup choreography. The early-window demand (w h0 590KB +
            # v tb0 786KB inside the first group's 3.9us) exceeds what two
            # cold rings deliver (~130-150GB/s each until they ramp), so:
            # sync takes weights then bulk V, scalar takes the first two
            # nu-pair V chunks, and the gpsimd ring (first byte ~10.1us,
            # then ~200GB/s) takes the nu45 chunk needed at ~13.4us.
            # Sub-3.4us stalls don't reset the HAM throttle warmup.
            nc.sync.dma_start(w_sb[:, 0:768], w[:, 0:768])  # h0 nu0,1
            nc.scalar.dma_start(vts[0][:, 0:2, 0:8, :], v_in[0, :, 0:2, 0:8, :])
            nc.gpsimd.dma_start(vts[0][:, 3, 0:8, :], v_in[0, :, 3, 0:8, :])
            nc.sync.dma_start(w_sb[:, 768:2304], w[:, 768:2304])  # h0 nu2-5
            nc.gpsimd.dma_start(vts[0][:, 4, 0:8, :], v_in[0, :, 4, 0:8, :])
            nc.sync.dma_start(vts[0][:, 2, 0:8, :], v_in[0, :, 2, 0:8, :])
            nc.gpsimd.dma_start(vts[0][:, 5, 0:8, :], v_in[0, :, 5, 0:8, :])
            nc.sync.dma_start(w_sb[:, 2304:3456], w[:, 2304:3456])  # h1 nu0-2
            nc.sync.dma_start(w_sb[:, 3456:4608], w[:, 3456:4608])  # h1 nu3-5
            nc.sync.dma_start(vts[0][:, :, 8:16, :], v_in[0, :, :, 8:16, :])
            nc.sync.dma_start(vts[1][:, :, 0:8, :], v_in[1, :, :, 0:8, :])
            nc.sync.dma_start(vts[1][:, :, 8:16, :], v_in[1, :, :, 8:16, :])

            def do_group(img, h, tb, last=False):
                t0, T = TBS[tb]
                v = vts[img]
                stg = s_pool.tile([128, NNU, 8, OW], bf16, name="stg")
                for nu in range(NNU):
                    p = ps_pool.tile([128, 8, OW], f32, name="p", tag="p")
                    for kj in range(3):
                        wsl = w_sb[
                            :,
                            ((h * NNU + nu) * 3 + kj) * 128 :
                            ((h * NNU + nu) * 3 + kj) * 128 + 128,
                        ]
                        nc.tensor.matmul(
                            p[:, :T, :],
                            wsl,
                            v[:, nu, t0 : t0 + T, kj : kj + OW],
                            start=(kj == 0),
                            stop=(kj == 2),
                        )
                    # PSUM -> SBUF bf16; alternate engines so each keeps pace
                    # with the 3-MM step of the PE stream
                    eng = nc.sync if nu % 2 == 0 else nc.scalar
                    if nu % 2 == 0:
                        nc.vector.tensor_copy(stg[:, nu, :T, :], p[:, :T, :])
                    else:
                        nc.scalar.copy(stg[:, nu, :T, :], p[:, :T, :])
                    if last:
                        # per-nu stores so only ~1 small store trails the
                        # final matmul (a single big tail store is several us)
                        eng.dma_start(
                            m_out[img, h, :, tb, nu, :, :],
                            stg[:, nu, :T, :],
                        )
                if not last:
                    nc.scalar.dma_start(
                        m_out[img, h, :, tb, :, :, :], stg[:, :, :T, :]
                    )

            # tb-major, h-inner: each V block feeds two consecutive groups.
            for img in range(IMGS_PER_CORE):
                for tb in range(2):
                    for h in range(2):
                        last = img == IMGS_PER_CORE - 1 and tb == 1 and h == 1
                        do_group(img, h, tb, last=last)
    nc.compile()
    return nc


def _get_nc():
    if not _NC_CACHE:
        _NC_CACHE.append(_build())
    return _NC_CACHE[0]


def _pack_weights(weights):
    # [cout, cin, kh, kw] -> Wt[v,kj,o,c] -> [cin, (h, nu, kj, coutl)]
    import ml_dtypes

    wt = np.einsum("vk,ockj->vjoc", _G, weights.astype(np.float64))
    # reorder to [c, h, nu, kj, ol]
    wt = wt.reshape(NNU, 3, 2, 128, CIN)  # v, kj, h, ol, c
    wtr = np.transpose(wt, (4, 2, 0, 1, 3))  # c, h, v, kj, ol
    return np.ascontiguousarray(wtr.reshape(CIN, 2 * NNU * 3 * 128)).astype(
        ml_dtypes.bfloat16
    )


def _pack_v(x):
    # x [n, cin, 64, 64] f32 -> V [n, cin, 6, 16, 64] bf16
    import ml_dtypes

    idx = _STARTS[:, None] + np.arange(6)[None, :]  # (16, 6)
    xg = x[:, :, idx, :]  # (n, c, 16, 6, 64)
    v = np.einsum("vj,nctjw->ncvtw", _BT.astype(np.float32), xg)
    return np.ascontiguousarray(v).astype(ml_dtypes.bfloat16)


def _make_in_maps(x, weights):
    xv = _pack_v(np.ascontiguousarray(x, dtype=np.float32))
    w_l = _pack_weights(np.ascontiguousarray(weights, dtype=np.float32))
    return [
        {"v": xv[IMGS_PER_CORE * c : IMGS_PER_CORE * (c + 1)], "w": w_l}
        for c in range(N_CORES)
    ]


def _ldw_opt_patch():
    """No-op (kept for harness compat)."""
    import contextlib

    return contextlib.nullcontext()


def _combine(m_all):
    # m_all: [16, 2, 128, 2, 6, 8, 62] bf16 -> out [16, 256, 62, 62] f32
    m = np.asarray(m_all, dtype=np.float32)
    n = m.shape[0]
    # [n, h, c, tb, v, t8, u] -> [n, h*128+c, v, tb*8+t8, u]
    m = m.transpose(0, 1, 2, 4, 3, 5, 6).reshape(n, 256, NNU, NT, OW)
    y = np.einsum("rv,novtu->notru", _AT.astype(np.float32), m)  # n,o,t,r,u
    out = np.empty((n, COUT, OH, OW), dtype=np.float32)
    for t in range(NT):
        out[:, :, _STARTS[t] : _STARTS[t] + 4, :] = y[:, :, t, :, :]
    return out


def kernel(x, weights):
    from concourse.bass_utils import run_bass_kernel_spmd

    nc = _get_nc()
    in_maps = _make_in_maps(x, weights)
    res = run_bass_kernel_spmd(nc, in_maps, core_ids=list(range(N_CORES)))
    m_all = np.concatenate([r["m"] for r in res.results], axis=0)
    return _combine(m_all)


# revision 29
# speedup vs baseline: 1.0007x; 1.0007x over previous
"""3x3 valid conv via 1D Winograd F(4,3) along H, on 8 Trainium2 cores,
with the output (A^T) combine moved to the HOST.

x: (16, 128, 64, 64) f32, weights: (256, 128, 3, 3) f32
-> out: (16, 256, 62, 62) f32

Data-parallel, 2 images per core. Interpolation points {0, 1, -1, 1/2, -2, inf}
(chosen to minimize bf16 transform error; classic {0,±1,±2} overflows the
2e-2 gate). 16 row-tiles of 4 output rows (tile 15 overlaps: rows 58-61).

Per tile: 6 nu x 3 kj = 18 matmuls per 4 output rows (vs 12 per 2 rows for
F(2,3)) -> 25% less PE work, and the device does NO output combine at all:
each PSUM bank M[nu] is copied to SBUF as bf16 (DVE/Act alternating) and
DMA'd out; the host applies y = A^T M in f32. This removes the DVE
tensor_tensor wall (the F(2,3) kernel's combine trailed the PE stream) and
cuts store bytes to 6.1MB bf16.

V transform (B^T, host, f64->bf16) and G weight transform also on host.
Three HWDGE rings: sync = weights + bulk V, Act = first V chunks + group
stores, gpsimd = two mid-startup V chunks (its DSP compute stays idle).
8 fp32 prewarm matmuls bridge the PE seamlessly to the first real matmul:
any >1us PE gap during warmup restarts the ~5us HAM throttle warmup clock
(half-rate PE until then). M stores are group-contiguous in DRAM (5952B
per-partition elements); smaller DMA elements throttle the ring enough
that store backpressure through the staging pool paces the whole stream.
"""

import numpy as np

N_CORES = 8
IMGS_PER_CORE = 2
CIN = 128
COUT = 256
H = W = 64
OH = OW = 62
NT = 16            # row-tiles per image (4 output rows each; tile 15 at row 58)
NNU = 6
TBS = [(0, 8), (8, 8)]  # (t0, ntiles) blocks; one PSUM bank per nu holds 8 tiles

# F(4,3) with points {0, 1, -1, 1/2, -2, inf}
_AT = np.array(
    [
        [1.0, 1.0, 1.0, 1.0, 1.0, 0.0],
        [0.0, 1.0, -1.0, 0.5, -2.0, 0.0],
        [0.0, 1.0, 1.0, 0.25, 4.0, 0.0],
        [0.0, 1.0, -1.0, 0.125, -8.0, 1.0],
    ]
)
_G = np.array(
    [
        [1.0, 0.0, 0.0],
        [1 / 3, 1 / 3, 1 / 3],
        [-1 / 3, 1 / 3, -1 / 3],
        [-16 / 15, -8 / 15, -4 / 15],
        [1 / 15, -2 / 15, 4 / 15],
        [0.0, 0.0, 1.0],
    ]
)
_BT = np.array(
    [
        [1.0, -1.5, -2.0, 1.5, 1.0, 0.0],
        [0.0, -1.0, 0.5, 2.5, 1.0, 0.0],
        [0.0, 1.0, -2.5, 0.5, 1.0, 0.0],
        [0.0, -2.0, -1.0, 2.0, 1.0, 0.0],
        [0.0, 0.5, -1.0, -0.5, 1.0, 0.0],
        [0.0, 1.0, -1.5, -2.0, 1.5, 1.0],
    ]
)
_STARTS = np.array([0, 4, 8, 12, 16, 20, 24, 28, 32, 36, 40, 44, 48, 52, 56, 58])

_NC_CACHE = []


def _build():
    import concourse.bacc as bacc
    import concourse.mybir as mybir
    import concourse.tile as tile

    bf16 = mybir.dt.bfloat16
    f32 = mybir.dt.float32

    nc = bacc.Bacc("TRN2", target_bir_lowering=False, debug=False)
    # V layout: [img, cin, nu, t, col] (nu-major: per-nu chunks are 1024B
    # contiguous per partition -- 256B elements run ~10x slower on the rings)
    v_in = nc.dram_tensor(
        "v", [IMGS_PER_CORE, CIN, NNU, NT, W], bf16, kind="ExternalInput"
    ).ap()
    # w layout: [cin, (h, nu, kj, coutl)]
    w = nc.dram_tensor(
        "w", [CIN, 2 * NNU * 3 * 128], bf16, kind="ExternalInput"
    ).ap()
    # M out: [img, h, coutl, tb, nu, t8, col] -- group-contiguous so each
    # group store is a single 5952B-per-partition element (992B elements
    # run the ring at ~140-220GB/s depending on luck and can gate the
    # whole stream through staging-buffer recycling; 5952B sustains ~400)
    m_out = nc.dram_tensor(
        "m", [IMGS_PER_CORE, 2, 128, 2, NNU, 8, OW], bf16, kind="ExternalOutput"
    ).ap()

    with tile.TileContext(nc) as tc:
        with (
            tc.tile_pool(name="wp", bufs=1) as w_pool,
            tc.tile_pool(name="vp", bufs=1) as v_pool,
            tc.tile_pool(name="sp", bufs=5) as s_pool,
            tc.tile_pool(name="ps", bufs=8, space="PSUM") as ps_pool,
        ):
            # PE prewarm through the initial DMA window. memset on DVE, NOT
            # gpsimd (gpsimd boots late and tightens the HAM throttle).
            scr = w_pool.tile([CIN, 128], f32, tag="scr")
            nc.vector.memset(scr[:], 0.0)
            wrm = ps_pool.tile([128, 8, OW], f32, name="wrm", tag="p")
            for _ in range(8):
                nc.tensor.matmul(
                    wrm[:, :2, :], scr[:], scr[:, :124], start=True, stop=True
                )

            w_sb = w_pool.tile([CIN, 2 * NNU * 3 * 128], bf16, tag="w")
            vts = {
                0: v_pool.tile([CIN, NNU, NT, W], bf16, name="v0", tag="v0"),
                1: v_pool.tile([CIN, NNU, NT, W], bf16, name="v1", tag="v1"),
            }
            # Startup choreography. The early-window demand (w h0 590KB +
            # v tb0 786KB inside the first group's 3.9us) exceeds what two
            # cold rings deliver (~130-150GB/s each until they ramp), so:
            # sync takes weights then bulk V, scalar takes the first two
            # nu-pair V chunks, and the gpsimd ring (first byte ~10.1us,
            # then ~200GB/s) takes the nu45 chunk needed at ~13.4us.
            # Sub-3.4us stalls don't reset the HAM throttle warmup.
            nc.sync.dma_start(w_sb[:, 0:768], w[:, 0:768])  # h0 nu0,1
            nc.scalar.dma_start(vts[0][:, 0:2, 0:8, :], v_in[0, :, 0:2, 0:8, :])
            nc.gpsimd.dma_start(vts[0][:, 3, 0:8, :], v_in[0, :, 3, 0:8, :])
            nc.sync.dma_start(w_sb[:, 768:2304], w[:, 768:2304])  # h0 nu2-5
            nc.scalar.dma_start(vts[0][:, 2, 0:8, :], v_in[0, :, 2, 0:8, :])
            nc.gpsimd.dma_start(vts[0][:, 4:6, 0:8, :], v_in[0, :, 4:6, 0:8, :])
            nc.sync.dma_start(w_sb[:, 2304:3456], w[:, 2304:3456])  # h1 nu0-2
            nc.sync.dma_start(w_sb[:, 3456:4608], w[:, 3456:4608])  # h1 nu3-5
            nc.sync.dma_start(vts[0][:, :, 8:16, :], v_in[0, :, :, 8:16, :])
            nc.sync.dma_start(vts[1][:, :, 0:8, :], v_in[1, :, :, 0:8, :])
            nc.sync.dma_start(vts[1][:, :, 8:16, :], v_in[1, :, :, 8:16, :])

            def do_group(img, h, tb, last=False, nu_order=None):
                t0, T = TBS[tb]
                v = vts[img]
                stg = s_pool.tile([128, NNU, 8, OW], bf16, name="stg")
                for nu in (nu_order or range(NNU)):
                    p = ps_pool.tile([128, 8, OW], f32, name="p", tag="p")
                    for kj in range(3):
                        wsl = w_sb[
                            :,
                            ((h * NNU + nu) * 3 + kj) * 128 :
                            ((h * NNU + nu) * 3 + kj) * 128 + 128,
                        ]
                        nc.tensor.matmul(
                            p[:, :T, :],
                            wsl,
                            v[:, nu, t0 : t0 + T, kj : kj + OW],
                            start=(kj == 0),
                            stop=(kj == 2),
                        )
                    # PSUM -> SBUF bf16; alternate engines so each keeps pace
                    # with the 3-MM step of the PE stream
                    eng = nc.sync if nu % 2 == 0 else nc.scalar
                    if nu % 2 == 0:
                        nc.vector.tensor_copy(stg[:, nu, :T, :], p[:, :T, :])
                    else:
                        nc.scalar.copy(stg[:, nu, :T, :], p[:, :T, :])
                    if last:
                        # per-nu stores so only ~1 small store trails the
                        # final matmul (a single big tail store is several us)
                        eng.dma_start(
                            m_out[img, h, :, tb, nu, :, :],
                            stg[:, nu, :T, :],
                        )
                if not last:
                    nc.scalar.dma_start(
                        m_out[img, h, :, tb, :, :, :], stg[:, :, :T, :]
                    )

            # tb-major, h-inner: each V block feeds two consecutive groups.
            # group 0 consumes nu2 LAST: its chunk is the latest arrival
            # of the startup choreography (~13.4us vs a +1.3us need slot);
            # deferring it to the +3.3us slot clears the final early stall
            # without touching the DMA schedule. Later groups have all data
            # resident, so order is irrelevant there.
            for img in range(IMGS_PER_CORE):
                for tb in range(2):
                    for h in range(2):
                        first = img == 0 and tb == 0 and h == 0
                        last = img == IMGS_PER_CORE - 1 and tb == 1 and h == 1
                        do_group(img, h, tb, last=last,
                                 nu_order=(0, 1, 3, 4, 5, 2) if first else None)
    nc.compile()
    return nc


def _get_nc():
    if not _NC_CACHE:
        _NC_CACHE.append(_build())
    return _NC_CACHE[0]


def _pack_weights(weights):
    # [cout, cin, kh, kw] -> Wt[v,kj,o,c] -> [cin, (h, nu, kj, coutl)]
    import ml_dtypes

    wt = np.einsum("vk,ockj->vjoc", _G, weights.astype(np.float64))
    # reorder to [c, h, nu, kj, ol]
    wt = wt.reshape(NNU, 3, 2, 128, CIN)  # v, kj, h, ol, c
    wtr = np.transpose(wt, (4, 2, 0, 1, 3))  # c, h, v, kj, ol
    return np.ascontiguousarray(wtr.reshape(CIN, 2 * NNU * 3 * 128)).astype(
        ml_dtypes.bfloat16
    )


def _pack_v(x):
    # x [n, cin, 64, 64] f32 -> V [n, cin, 6, 16, 64] bf16
    import ml_dtypes

    idx = _STARTS[:, None] + np.arange(6)[None, :]  # (16, 6)
    xg = x[:, :, idx, :]  # (n, c, 16, 6, 64)
    v = np.einsum("vj,nctjw->ncvtw", _BT.astype(np.float32), xg)
    return np.ascontiguousarray(v).astype(ml_dtypes.bfloat16)


def _make_in_maps(x, weights):
    xv = _pack_v(np.ascontiguousarray(x, dtype=np.float32))
    w_l = _pack_weights(np.ascontiguousarray(weights, dtype=np.float32))
    return [
        {"v": xv[IMGS_PER_CORE * c : IMGS_PER_CORE * (c + 1)], "w": w_l}
        for c in range(N_CORES)
    ]


def _ldw_opt_patch():
    """No-op (kept for harness compat)."""
    import contextlib

    return contextlib.nullcontext()


def _combine(m_all):
    # m_all: [16, 2, 128, 2, 6, 8, 62] bf16 -> out [16, 256, 62, 62] f32
    m = np.asarray(m_all, dtype=np.float32)
    n = m.shape[0]
    # [n, h, c, tb, v, t8, u] -> [n, h*128+c, v, tb*8+t8, u]
    m = m.transpose(0, 1, 2, 4, 3, 5, 6).reshape(n, 256, NNU, NT, OW)
    y = np.einsum("rv,novtu->notru", _AT.astype(np.float32), m)  # n,o,t,r,u
    out = np.empty((n, COUT, OH, OW), dtype=np.float32)
    for t in range(NT):
        out[:, :, _STARTS[t] : _STARTS[t] + 4, :] = y[:, :, t, :, :]
    return out


def kernel(x, weights):
    from concourse.bass_utils import run_bass_kernel_spmd

    nc = _get_nc()
    in_maps = _make_in_maps(x, weights)
    res = run_bass_kernel_spmd(nc, in_maps, core_ids=list(range(N_CORES)))
    m_all = np.concatenate([r["m"] for r in res.results], axis=0)
    return _combine(m_all)


# revision 30
# speedup vs baseline: 1.0008x; 1.0001x over previous
"""3x3 valid conv via 1D Winograd F(4,3) along H, on 8 Trainium2 cores,
with the output (A^T) combine moved to the HOST.

x: (16, 128, 64, 64) f32, weights: (256, 128, 3, 3) f32
-> out: (16, 256, 62, 62) f32

Data-parallel, 2 images per core. Interpolation points {0, 1, -1, 1/2, -2, inf}
(chosen to minimize bf16 transform error; classic {0,±1,±2} overflows the
2e-2 gate). 16 row-tiles of 4 output rows (tile 15 overlaps: rows 58-61).

Per tile: 6 nu x 3 kj = 18 matmuls per 4 output rows (vs 12 per 2 rows for
F(2,3)) -> 25% less PE work, and the device does NO output combine at all:
each PSUM bank M[nu] is copied to SBUF as bf16 (DVE/Act alternating) and
DMA'd out; the host applies y = A^T M in f32. This removes the DVE
tensor_tensor wall (the F(2,3) kernel's combine trailed the PE stream) and
cuts store bytes to 6.1MB bf16.

V transform (B^T, host, f64->bf16) and G weight transform also on host.
Three HWDGE rings: sync = weights + bulk V, Act = first V chunks + group
stores, gpsimd = two mid-startup V chunks (its DSP compute stays idle).
8 fp32 prewarm matmuls bridge the PE seamlessly to the first real matmul:
any >1us PE gap during warmup restarts the ~5us HAM throttle warmup clock
(half-rate PE until then). M stores are group-contiguous in DRAM (5952B
per-partition elements); smaller DMA elements throttle the ring enough
that store backpressure through the staging pool paces the whole stream.
"""

import numpy as np

N_CORES = 8
IMGS_PER_CORE = 2
CIN = 128
COUT = 256
H = W = 64
OH = OW = 62
NT = 16            # row-tiles per image (4 output rows each; tile 15 at row 58)
NNU = 6
TBS = [(0, 8), (8, 8)]  # (t0, ntiles) blocks; one PSUM bank per nu holds 8 tiles

# F(4,3) with points {0, 1, -1, 1/2, -2, inf}
_AT = np.array(
    [
        [1.0, 1.0, 1.0, 1.0, 1.0, 0.0],
        [0.0, 1.0, -1.0, 0.5, -2.0, 0.0],
        [0.0, 1.0, 1.0, 0.25, 4.0, 0.0],
        [0.0, 1.0, -1.0, 0.125, -8.0, 1.0],
    ]
)
_G = np.array(
    [
        [1.0, 0.0, 0.0],
        [1 / 3, 1 / 3, 1 / 3],
        [-1 / 3, 1 / 3, -1 / 3],
        [-16 / 15, -8 / 15, -4 / 15],
        [1 / 15, -2 / 15, 4 / 15],
        [0.0, 0.0, 1.0],
    ]
)
_BT = np.array(
    [
        [1.0, -1.5, -2.0, 1.5, 1.0, 0.0],
        [0.0, -1.0, 0.5, 2.5, 1.0, 0.0],
        [0.0, 1.0, -2.5, 0.5, 1.0, 0.0],
        [0.0, -2.0, -1.0, 2.0, 1.0, 0.0],
        [0.0, 0.5, -1.0, -0.5, 1.0, 0.0],
        [0.0, 1.0, -1.5, -2.0, 1.5, 1.0],
    ]
)
_STARTS = np.array([0, 4, 8, 12, 16, 20, 24, 28, 32, 36, 40, 44, 48, 52, 56, 58])

_NC_CACHE = []


def _build():
    import concourse.bacc as bacc
    import concourse.mybir as mybir
    import concourse.tile as tile

    bf16 = mybir.dt.bfloat16
    f32 = mybir.dt.float32

    nc = bacc.Bacc("TRN2", target_bir_lowering=False, debug=False)
    # V layout: [img, cin, nu, t, col] (nu-major: per-nu chunks are 1024B
    # contiguous per partition -- 256B elements run ~10x slower on the rings)
    v_in = nc.dram_tensor(
        "v", [IMGS_PER_CORE, CIN, NNU, NT, W], bf16, kind="ExternalInput"
    ).ap()
    # w layout: [cin, (h, nu, kj, coutl)]
    w = nc.dram_tensor(
        "w", [CIN, 2 * NNU * 3 * 128], bf16, kind="ExternalInput"
    ).ap()
    # M out: [img, h, coutl, tb, nu, t8, col] -- group-contiguous so each
    # group store is a single 5952B-per-partition element (992B elements
    # run the ring at ~140-220GB/s depending on luck and can gate the
    # whole stream through staging-buffer recycling; 5952B sustains ~400)
    m_out = nc.dram_tensor(
        "m", [IMGS_PER_CORE, 2, 128, 2, NNU, 8, OW], bf16, kind="ExternalOutput"
    ).ap()

    with tile.TileContext(nc) as tc:
        with (
            tc.tile_pool(name="wp", bufs=1) as w_pool,
            tc.tile_pool(name="vp", bufs=1) as v_pool,
            tc.tile_pool(name="sp", bufs=5) as s_pool,
            tc.tile_pool(name="ps", bufs=8, space="PSUM") as ps_pool,
        ):
            # PE prewarm through the initial DMA window. memset on DVE, NOT
            # gpsimd (gpsimd boots late and tightens the HAM throttle).
            scr = w_pool.tile([CIN, 128], f32, tag="scr")
            nc.vector.memset(scr[:], 0.0)
            wrm = ps_pool.tile([128, 8, OW], f32, name="wrm", tag="p")
            for _ in range(8):
                nc.tensor.matmul(
                    wrm[:, :2, :], scr[:], scr[:, :124], start=True, stop=True
                )

            w_sb = w_pool.tile([CIN, 2 * NNU * 3 * 128], bf16, tag="w")
            vts = {
                0: v_pool.tile([CIN, NNU, NT, W], bf16, name="v0", tag="v0"),
                1: v_pool.tile([CIN, NNU, NT, W], bf16, name="v1", tag="v1"),
            }
            # Startup choreography. The early-window demand (w h0 590KB +
            # v tb0 786KB inside the first group's 3.9us) exceeds what two
            # cold rings deliver (~130-150GB/s each until they ramp), so:
            # sync takes weights then bulk V, scalar takes the first two
            # nu-pair V chunks, and the gpsimd ring (first byte ~10.1us,
            # then ~200GB/s) takes the nu45 chunk needed at ~13.4us.
            # Sub-3.4us stalls don't reset the HAM throttle warmup.
            nc.sync.dma_start(w_sb[:, 0:768], w[:, 0:768])  # h0 nu0,1
            nc.scalar.dma_start(vts[0][:, 0:2, 0:8, :], v_in[0, :, 0:2, 0:8, :])
            nc.gpsimd.dma_start(vts[0][:, 3, 0:8, :], v_in[0, :, 3, 0:8, :])
            nc.sync.dma_start(w_sb[:, 768:2304], w[:, 768:2304])  # h0 nu2-5
            nc.scalar.dma_start(vts[0][:, 2, 0:8, :], v_in[0, :, 2, 0:8, :])
            nc.gpsimd.dma_start(vts[0][:, 4:6, 0:8, :], v_in[0, :, 4:6, 0:8, :])
            nc.sync.dma_start(w_sb[:, 2304:3456], w[:, 2304:3456])  # h1 nu0-2
            nc.sync.dma_start(w_sb[:, 3456:4608], w[:, 3456:4608])  # h1 nu3-5
            nc.sync.dma_start(vts[0][:, :, 8:16, :], v_in[0, :, :, 8:16, :])
            nc.sync.dma_start(vts[1][:, :, 0:8, :], v_in[1, :, :, 0:8, :])
            nc.sync.dma_start(vts[1][:, :, 8:16, :], v_in[1, :, :, 8:16, :])

            def do_group(img, h, tb, last=False):
                t0, T = TBS[tb]
                v = vts[img]
                stg = s_pool.tile([128, NNU, 8, OW], bf16, name="stg")
                for nu in range(NNU):
                    p = ps_pool.tile([128, 8, OW], f32, name="p", tag="p")
                    for kj in range(3):
                        wsl = w_sb[
                            :,
                            ((h * NNU + nu) * 3 + kj) * 128 :
                            ((h * NNU + nu) * 3 + kj) * 128 + 128,
                        ]
                        nc.tensor.matmul(
                            p[:, :T, :],
                            wsl,
                            v[:, nu, t0 : t0 + T, kj : kj + OW],
                            start=(kj == 0),
                            stop=(kj == 2),
                        )
                    # PSUM -> SBUF bf16; alternate engines so each keeps pace
                    # with the 3-MM step of the PE stream
                    eng = nc.sync if nu % 2 == 0 else nc.scalar
                    if nu % 2 == 0:
                        nc.vector.tensor_copy(stg[:, nu, :T, :], p[:, :T, :])
                    else:
                        nc.scalar.copy(stg[:, nu, :T, :], p[:, :T, :])
                    if last:
                        # per-nu stores so only ~1 small store trails the
                        # final matmul (a single big tail store is several us)
                        eng.dma_start(
                            m_out[img, h, :, tb, nu, :, :],
                            stg[:, nu, :T, :],
                        )
                if not last:
                    nc.scalar.dma_start(
                        m_out[img, h, :, tb, :, :, :], stg[:, :, :T, :]
                    )

            # tb-major, h-inner: each V block feeds two consecutive groups.
            for img in range(IMGS_PER_CORE):
                for tb in range(2):
                    for h in range(2):
                        last = img == IMGS_PER_CORE - 1 and tb == 1 and h == 1
                        do_group(img, h, tb, last=last)
    nc.compile()
    return nc


def _get_nc():
    if not _NC_CACHE:
        _NC_CACHE.append(_build())
    return _NC_CACHE[0]


def _pack_weights(weights):
    # [cout, cin, kh, kw] -> Wt[v,kj,o,c] -> [cin, (h, nu, kj, coutl)]
    import ml_dtypes

    wt = np.einsum("vk,ockj->vjoc", _G, weights.astype(np.float64))
    # reorder to [c, h, nu, kj, ol]
    wt = wt.reshape(NNU, 3, 2, 128, CIN)  # v, kj, h, ol, c
    wtr = np.transpose(wt, (4, 2, 0, 1, 3))  # c, h, v, kj, ol
    return np.ascontiguousarray(wtr.reshape(CIN, 2 * NNU * 3 * 128)).astype(
        ml_dtypes.bfloat16
    )


def _pack_v(x):
    # x [n, cin, 64, 64] f32 -> V [n, cin, 6, 16, 64] bf16
    import ml_dtypes

    idx = _STARTS[:, None] + np.arange(6)[None, :]  # (16, 6)
    xg = x[:, :, idx, :]  # (n, c, 16, 6, 64)
    v = np.einsum("vj,nctjw->ncvtw", _BT.astype(np.float32), xg)
    return np.ascontiguousarray(v).astype(ml_dtypes.bfloat16)


def _make_in_maps(x, weights):
    xv = _pack_v(np.ascontiguousarray(x, dtype=np.float32))
    w_l = _pack_weights(np.ascontiguousarray(weights, dtype=np.float32))
    return [
        {"v": xv[IMGS_PER_CORE * c : IMGS_PER_CORE * (c + 1)], "w": w_l}
        for c in range(N_CORES)
    ]


def _ldw_opt_patch():
    """No-op (kept for harness compat)."""
    import contextlib

    return contextlib.nullcontext()


def _combine(m_all):
    # m_all: [16, 2, 128, 2, 6, 8, 62] bf16 -> out [16, 256, 62, 62] f32
    m = np.asarray(m_all, dtype=np.float32)
    n = m.shape[0]
    # [n, h, c, tb, v, t8, u] -> [n, h*128+c, v, tb*8+t8, u]
    m = m.transpose(0, 1, 2, 4, 3, 5, 6).reshape(n, 256, NNU, NT, OW)
    y = np.einsum("rv,novtu->notru", _AT.astype(np.float32), m)  # n,o,t,r,u
    out = np.empty((n, COUT, OH, OW), dtype=np.float32)
    for t in range(NT):
        out[:, :, _STARTS[t] : _STARTS[t] + 4, :] = y[:, :, t, :, :]
    return out


def kernel(x, weights):
    from concourse.bass_utils import run_bass_kernel_spmd

    nc = _get_nc()
    in_maps = _make_in_maps(x, weights)
    res = run_bass_kernel_spmd(nc, in_maps, core_ids=list(range(N_CORES)))
    m_all = np.concatenate([r["m"] for r in res.results], axis=0)
    return _combine(m_all)


# revision 31
# speedup vs baseline: 1.0179x; 1.0171x over previous
"""3x3 valid conv via 1D Winograd F(4,3) along H, on 8 Trainium2 cores,
with the output (A^T) combine moved to the HOST.

x: (16, 128, 64, 64) f32, weights: (256, 128, 3, 3) f32
-> out: (16, 256, 62, 62) f32

Data-parallel, 2 images per core. Interpolation points {0, 1, -1, 1/2, -2, inf}
(chosen to minimize bf16 transform error; classic {0,±1,±2} overflows the
2e-2 gate). 16 row-tiles of 4 output rows (tile 15 overlaps: rows 58-61).

Per tile: 6 nu x 3 kj = 18 matmuls per 4 output rows (vs 12 per 2 rows for
F(2,3)) -> 25% less PE work, and the device does NO output combine at all:
each PSUM bank M[nu] is copied to SBUF as bf16 (DVE/Act alternating) and
DMA'd out; the host applies y = A^T M in f32. This removes the DVE
tensor_tensor wall (the F(2,3) kernel's combine trailed the PE stream) and
cuts store bytes to 6.1MB bf16.

V transform (B^T, host, f64->bf16) and G weight transform also on host.
Three HWDGE rings: sync = weights + bulk V, Act = first V chunks + group
stores, gpsimd = two mid-startup V chunks (its DSP compute stays idle).
8 fp32 prewarm matmuls bridge the PE seamlessly to the first real matmul:
any >1us PE gap during warmup restarts the ~5us HAM throttle warmup clock
(half-rate PE until then). M stores are group-contiguous in DRAM (5952B
per-partition elements); smaller DMA elements throttle the ring enough
that store backpressure through the staging pool paces the whole stream.
"""

import numpy as np

N_CORES = 8
IMGS_PER_CORE = 2
CIN = 128
COUT = 256
H = W = 64
OH = OW = 62
NT = 16            # row-tiles per image (4 output rows each; tile 15 at row 58)
NNU = 6
TBS = [(0, 8), (8, 8)]  # (t0, ntiles) blocks; one PSUM bank per nu holds 8 tiles

# F(4,3) with points {0, 1, -1, 1/2, -2, inf}
_AT = np.array(
    [
        [1.0, 1.0, 1.0, 1.0, 1.0, 0.0],
        [0.0, 1.0, -1.0, 0.5, -2.0, 0.0],
        [0.0, 1.0, 1.0, 0.25, 4.0, 0.0],
        [0.0, 1.0, -1.0, 0.125, -8.0, 1.0],
    ]
)
_G = np.array(
    [
        [1.0, 0.0, 0.0],
        [1 / 3, 1 / 3, 1 / 3],
        [-1 / 3, 1 / 3, -1 / 3],
        [-16 / 15, -8 / 15, -4 / 15],
        [1 / 15, -2 / 15, 4 / 15],
        [0.0, 0.0, 1.0],
    ]
)
_BT = np.array(
    [
        [1.0, -1.5, -2.0, 1.5, 1.0, 0.0],
        [0.0, -1.0, 0.5, 2.5, 1.0, 0.0],
        [0.0, 1.0, -2.5, 0.5, 1.0, 0.0],
        [0.0, -2.0, -1.0, 2.0, 1.0, 0.0],
        [0.0, 0.5, -1.0, -0.5, 1.0, 0.0],
        [0.0, 1.0, -1.5, -2.0, 1.5, 1.0],
    ]
)
_STARTS = np.array([0, 4, 8, 12, 16, 20, 24, 28, 32, 36, 40, 44, 48, 52, 56, 58])

_NC_CACHE = []


def _build():
    import concourse.bacc as bacc
    import concourse.mybir as mybir
    import concourse.tile as tile

    bf16 = mybir.dt.bfloat16
    f32 = mybir.dt.float32

    nc = bacc.Bacc("TRN2", target_bir_lowering=False, debug=False)
    # V layout: [img, cin, nu, t, col] (nu-major: per-nu chunks are 1024B
    # contiguous per partition -- 256B elements run ~10x slower on the rings)
    v_in = nc.dram_tensor(
        "v", [IMGS_PER_CORE, CIN, NNU, NT, W], bf16, kind="ExternalInput"
    ).ap()
    # w layout: [cin, (h, nu, kj, coutl)]
    w = nc.dram_tensor(
        "w", [CIN, 2 * NNU * 3 * 128], bf16, kind="ExternalInput"
    ).ap()
    # M out: [img, h, coutl, tb, nu, t8, col] -- group-contiguous so each
    # group store is a single 5952B-per-partition element (992B elements
    # run the ring at ~140-220GB/s depending on luck and can gate the
    # whole stream through staging-buffer recycling; 5952B sustains ~400)
    m_out = nc.dram_tensor(
        "m", [IMGS_PER_CORE, 2, 128, 2, NNU, 8, OW], bf16, kind="ExternalOutput"
    ).ap()

    with tile.TileContext(nc) as tc:
        with (
            tc.tile_pool(name="wp", bufs=1) as w_pool,
            tc.tile_pool(name="vp", bufs=1) as v_pool,
            tc.tile_pool(name="sp", bufs=5) as s_pool,
            tc.tile_pool(name="ps", bufs=8, space="PSUM") as ps_pool,
        ):
            # PE prewarm through the initial DMA window. memset on DVE, NOT
            # gpsimd (gpsimd boots late and tightens the HAM throttle).
            scr = w_pool.tile([CIN, 128], f32, tag="scr")
            nc.vector.memset(scr[:], 0.0)
            wrm = ps_pool.tile([128, 8, OW], f32, name="wrm", tag="p")
            for _ in range(9):
                nc.tensor.matmul(
                    wrm[:, :2, :], scr[:], scr[:, :124], start=True, stop=True
                )

            w_sb = w_pool.tile([CIN, 2 * NNU * 3 * 128], bf16, tag="w")
            vts = {
                0: v_pool.tile([CIN, NNU, NT, W], bf16, name="v0", tag="v0"),
                1: v_pool.tile([CIN, NNU, NT, W], bf16, name="v1", tag="v1"),
            }
            # Startup choreography. The early-window demand (w h0 590KB +
            # v tb0 786KB inside the first group's 3.9us) exceeds what two
            # cold rings deliver (~130-150GB/s each until they ramp), so:
            # sync takes weights then bulk V, scalar takes the first two
            # nu-pair V chunks, and the gpsimd ring (first byte ~10.1us,
            # then ~200GB/s) takes the nu45 chunk needed at ~13.4us.
            # Sub-3.4us stalls don't reset the HAM throttle warmup.
            nc.sync.dma_start(w_sb[:, 0:768], w[:, 0:768])  # h0 nu0,1
            nc.scalar.dma_start(vts[0][:, 0:2, 0:8, :], v_in[0, :, 0:2, 0:8, :])
            nc.gpsimd.dma_start(vts[0][:, 3, 0:8, :], v_in[0, :, 3, 0:8, :])
            nc.sync.dma_start(w_sb[:, 768:2304], w[:, 768:2304])  # h0 nu2-5
            nc.scalar.dma_start(vts[0][:, 2, 0:8, :], v_in[0, :, 2, 0:8, :])
            nc.gpsimd.dma_start(vts[0][:, 4:6, 0:8, :], v_in[0, :, 4:6, 0:8, :])
            nc.sync.dma_start(w_sb[:, 2304:3456], w[:, 2304:3456])  # h1 nu0-2
            nc.sync.dma_start(w_sb[:, 3456:4608], w[:, 3456:4608])  # h1 nu3-5
            nc.sync.dma_start(vts[0][:, :, 8:16, :], v_in[0, :, :, 8:16, :])
            nc.sync.dma_start(vts[1][:, :, 0:8, :], v_in[1, :, :, 0:8, :])
            nc.sync.dma_start(vts[1][:, :, 8:16, :], v_in[1, :, :, 8:16, :])

            def do_group(img, h, tb, last=False):
                t0, T = TBS[tb]
                v = vts[img]
                stg = s_pool.tile([128, NNU, 8, OW], bf16, name="stg")
                for nu in range(NNU):
                    p = ps_pool.tile([128, 8, OW], f32, name="p", tag="p")
                    for kj in range(3):
                        wsl = w_sb[
                            :,
                            ((h * NNU + nu) * 3 + kj) * 128 :
                            ((h * NNU + nu) * 3 + kj) * 128 + 128,
                        ]
                        nc.tensor.matmul(
                            p[:, :T, :],
                            wsl,
                            v[:, nu, t0 : t0 + T, kj : kj + OW],
                            start=(kj == 0),
                            stop=(kj == 2),
                        )
                    # PSUM -> SBUF bf16; alternate engines so each keeps pace
                    # with the 3-MM step of the PE stream
                    eng = nc.sync if nu % 2 == 0 else nc.scalar
                    if nu % 2 == 0:
                        nc.vector.tensor_copy(stg[:, nu, :T, :], p[:, :T, :])
                    else:
                        nc.scalar.copy(stg[:, nu, :T, :], p[:, :T, :])
                    if last:
                        # per-nu stores so only ~1 small store trails the
                        # final matmul (a single big tail store is several us)
                        eng.dma_start(
                            m_out[img, h, :, tb, nu, :, :],
                            stg[:, nu, :T, :],
                        )
                if not last:
                    nc.scalar.dma_start(
                        m_out[img, h, :, tb, :, :, :], stg[:, :, :T, :]
                    )

            # tb-major, h-inner: each V block feeds two consecutive groups.
            for img in range(IMGS_PER_CORE):
                for tb in range(2):
                    for h in range(2):
                        last = img == IMGS_PER_CORE - 1 and tb == 1 and h == 1
                        do_group(img, h, tb, last=last)
    nc.compile()
    return nc


def _get_nc():
    if not _NC_CACHE:
        _NC_CACHE.append(_build())
    return _NC_CACHE[0]


def _pack_weights(weights):
    # [cout, cin, kh, kw] -> Wt[v,kj,o,c] -> [cin, (h, nu, kj, coutl)]
    import ml_dtypes

    wt = np.einsum("vk,ockj->vjoc", _G, weights.astype(np.float64))
    # reorder to [c, h, nu, kj, ol]
    wt = wt.reshape(NNU, 3, 2, 128, CIN)  # v, kj, h, ol, c
    wtr = np.transpose(wt, (4, 2, 0, 1, 3))  # c, h, v, kj, ol
    return np.ascontiguousarray(wtr.reshape(CIN, 2 * NNU * 3 * 128)).astype(
        ml_dtypes.bfloat16
    )


def _pack_v(x):
    # x [n, cin, 64, 64] f32 -> V [n, cin, 6, 16, 64] bf16
    import ml_dtypes

    idx = _STARTS[:, None] + np.arange(6)[None, :]  # (16, 6)
    xg = x[:, :, idx, :]  # (n, c, 16, 6, 64)
    v = np.einsum("vj,nctjw->ncvtw", _BT.astype(np.float32), xg)
    return np.ascontiguousarray(v).astype(ml_dtypes.bfloat16)


def _make_in_maps(x, weights):
    xv = _pack_v(np.ascontiguousarray(x, dtype=np.float32))
    w_l = _pack_weights(np.ascontiguousarray(weights, dtype=np.float32))
    return [
        {"v": xv[IMGS_PER_CORE * c : IMGS_PER_CORE * (c + 1)], "w": w_l}
        for c in range(N_CORES)
    ]


def _ldw_opt_patch():
    """No-op (kept for harness compat)."""
    import contextlib

    return contextlib.nullcontext()


def _combine(m_all):
    # m_all: [16, 2, 128, 2, 6, 8, 62] bf16 -> out [16, 256, 62, 62] f32
    m = np.asarray(m_all, dtype=np.float32)
    n = m.shape[0]
    # [n, h, c, tb, v, t8, u] -> [n, h*128+c, v, tb*8+t8, u]
    m = m.transpose(0, 1, 2, 4, 3, 5, 6).reshape(n, 256, NNU, NT, OW)
    y = np.einsum("rv,novtu->notru", _AT.astype(np.float32), m)  # n,o,t,r,u
    out = np.empty((n, COUT, OH, OW), dtype=np.float32)
    for t in range(NT):
        out[:, :, _STARTS[t] : _STARTS[t] + 4, :] = y[:, :, t, :, :]
    return out


def kernel(x, weights):
    from concourse.bass_utils import run_bass_kernel_spmd

    nc = _get_nc()
    in_maps = _make_in_maps(x, weights)
    res = run_bass_kernel_spmd(nc, in_maps, core_ids=list(range(N_CORES)))
    m_all = np.concatenate([r["m"] for r in res.results], axis=0)
    return _combine(m_all)
